# revision 9
# baseline (speedup 1.0000x reference)
"""MinGRU cell kernel for Trainium2 (8 NeuronCores, data-parallel over batch).

Computes, for x:[B,T,D], motion_mag:[B,T]:
    tau = 1 + softplus(alpha) * sigmoid(mw*mm + mb)        (per b,t)
    z   = sigmoid((x @ Wz^T + bz) / tau)                   (B,T,H)
    q   = x @ Wh^T (+ bh)                                  (B,T,H)
    h_t = (1-z_t)*h_{t-1} + z_t*q_t   (scan over t, h_0=0)

Strategy (vs the stock-op baseline at ~182us):
  - bf16 matmuls and bf16 HBM I/O: halves DMA traffic and SBUF footprint;
    PE rate is identical to fp32r (1 col/cycle). End-to-end rel err ~4e-3
    (gate 2e-2).
  - The whole elementwise recurrence tail is ONE custom DVE instruction
    (MINGRU_SCAN_ANT, hand-written uOp program): it consumes the gate z
    (SBUF bf16) and the candidate q (PSUM fp32) directly and computes
    h = (1-z)*h_prev + z*q at 1 element/cycle by interleaving the two
    hc-chunk streams of an hc-pair (stock tensor_tensor_scan runs at 2
    cycles/element and needs two extra DVE passes + an extra ACT pass to
    form (1-z) and z*q). DVE busy drops ~146us -> ~75us.
  - bh is folded out via h' = h - bh: the scan runs on raw q with initial
    carry -bh, and bh is re-added on the host (bh==0 here, so it's free).
  - Per (b, tb, pair) group: 16 z-matmuls -> one 4-bank PSUM tile, 2
    STT-u (u=(zpre+bz)*invtau, written column-interleaved), 1 sigmoid
    over the interleaved pair, 16 q-matmuls -> second 4-bank PSUM tile,
    1 fused scan, 1 out-DMA. Two 4-bank PSUM tiles ping-pong.
"""

import sys

import numpy as np

if "/opt/trn_rl_repo" not in sys.path:
    sys.path.insert(0, "/opt/trn_rl_repo")

import ml_dtypes

B, T, D, H = 32, 2048, 512, 512
NCORES = 8
BL = B // NCORES            # batch per core = 4
TBLK = 1024                 # t-columns per group
NTB = T // TBLK             # 2 t-blocks per sample
MMN = 512                   # matmul free-dim (1 psum bank)
DC = D // 128               # 4 contraction chunks
HC = H // 128               # 4 h partition chunks
PAIRS = HC // 2             # 2 hc-pairs (interleaved scan streams)
BT = BL * T                 # 8192 columns per core

_CACHE = {}

# --------------------------------------------------------------------------- #
# Custom DVE op: fused interleaved minGRU scan.
#
# Over a stream of 2*N elements interleaving two recurrences A (even k) and
# B (odd k):
#     h_k = (1 - z_k) * h_{k-2} + z_k * q_k     h_{-2}=s0[p], h_{-1}=s1[p]
#
# Datapath (8 stages, v3/TRN2):
#   inputs: lane0 = SRC_0 (q), lane1 = SRC_1 (z), lane2 = ONE_F32
#   stage 0: d = q * z           ; carry z (lane1), 1.0 (lane2)
#   stage 1: e = 1.0 - z         ; capture d -> lane3
#   stage 2: m = e * h_prev      ; h_prev via NEXT_ALU_OUT_A/B (stage-3 flop)
#   stage 3: h = m + d           ; a-flop (stream A) / b-flop (stream B)
#   stages 4-7: BYPASS chain to the write port.
#
# Each stream's running h lives in its own stage-3 flop, maintained by two
# alternating steady uOps (uOp transitions are zero-cost), which makes the
# recurrence immune to pipeline stalls: a flop holds its last value until
# the next element of the SAME stream rewrites it. Two 1-cycle
# non-consuming seed uOps preload the flops from CONST_0/CONST_1 (the
# [P,1] h-init carries). Measured: 2339ns per 2048-element instruction
# (1.14 cyc/elem); exact vs numpy in fp32.
# --------------------------------------------------------------------------- #


def _define_mingru_scan():
    from concourse.dve_ops import (
        OPS,
        _SUB_OPCODE_FOR_NAME,
        CUSTOM_DVE_SPECS,
        DveOp,
    )
    from concourse.dve_spec import C0, C1, Spec, Src0, Src1
    from concourse.dve_uop import (
        ENABLE,
        AluInp,
        AluOp,
        DelayInp,
        DveOpSpec,
        InpSel,
        OutPath,
        OutSel,
        Trigger,
        UopConfig,
    )

    name = "MINGRU_SCAN_ANT"
    if name in _SUB_OPCODE_FOR_NAME:
        return next(op for op in OPS if op.name == name)

    def _steady(stream_b, other_idx):
        u = UopConfig()
        u.enable_input(InpSel.SRC_0, 1)     # lane0 = q
        u.enable_input(InpSel.SRC_1, 2)     # lane1 = z
        u.enable_input(InpSel.ONE_F32, 3)   # lane2 = 1.0
        dp = u.datapath_config
        dp[0].enable_alu(AluOp.MULTIPLY, AluInp.PREV_DELAY_0, AluInp.PREV_DELAY_1)
        dp[0].pass_through_delay(1, 2)
        dp[1].enable_alu(AluOp.SUBTRACT, AluInp.PREV_DELAY_2, AluInp.PREV_DELAY_1)
        dp[1].enable_delay_from_src(DelayInp.PREV_ALU_OUT, 3)
        dp[2].enable_alu(
            AluOp.MULTIPLY,
            AluInp.PREV_ALU_OUT,
            AluInp.NEXT_ALU_OUT_B if stream_b else AluInp.NEXT_ALU_OUT_A,
        )
        dp[2].pass_through_delay(3)
        dp[3].enable_alu(AluOp.ADD, AluInp.PREV_ALU_OUT, AluInp.PREV_DELAY_3)
        if stream_b:
            dp[3].alu_out_b_enable = ENABLE
        else:
            dp[3].alu_out_a_enable = ENABLE
        for s in range(4, 8):
            dp[s].pass_through_alu()
        u.enable_output(OutSel.ALU_OUT, OutPath.WR0_LO)
        u.require_inp0 = ENABLE
        u.require_inp1 = ENABLE
        u.repeat_count = 1
        u.trigger = (Trigger.SRC_TENSOR_DONE, Trigger.COUNT, Trigger.NONE)
        u.next_uop = (0, other_idx, 0)
        return u

    def _seed(const, stream_b, next_idx):
        u = UopConfig()
        u.enable_input(const, 1)
        dp = u.datapath_config
        dp[0].enable_alu(AluOp.BYPASS, AluInp.PREV_DELAY_0, AluInp.PREV_DELAY_0)
        for s in range(1, 8):
            dp[s].pass_through_alu()
        if stream_b:
            dp[3].alu_out_b_enable = ENABLE
        else:
            dp[3].alu_out_a_enable = ENABLE
        u.repeat_count = 1
        u.trigger = (Trigger.COUNT, Trigger.NONE, Trigger.NONE)
        u.next_uop = (next_idx, 0, 0)
        return u

    def _reference(in0, in1, s0, s1, imm2):
        P = in0.shape[0]
        q = np.asarray(in0, np.float32).reshape(P, -1)
        z = np.asarray(in1, np.float32).reshape(P, -1)
        n2 = q.shape[1]
        h = np.empty((P, n2), np.float32)
        prev = [
            np.broadcast_to(np.asarray(s0, np.float32).reshape(-1), (P,)).copy(),
            np.broadcast_to(np.asarray(s1, np.float32).reshape(-1), (P,)).copy(),
        ]
        for k in range(n2):
            s = k & 1
            prev[s] = (1.0 - z[:, k]) * prev[s] + z[:, k] * q[:, k]
            h[:, k] = prev[s]
        return h

    class _HandWrittenDveOp(DveOp):
        def compile(self, ver):
            assert ver == "v3", f"{name} only authored for v3/TRN2, got {ver}"
            s = DveOpSpec(
                name=self.name,
                opcode=_SUB_OPCODE_FOR_NAME[self.name],
                uops=[
                    _seed(InpSel.CONST_0, False, 1),
                    _seed(InpSel.CONST_1, True, 2),
                    _steady(False, 3),
                    _steady(True, 2),
                ],
                rd1_en=True,
            )
            s.validate(ver)
            return s

    op = _HandWrittenDveOp(
        name,
        Spec(body=Src0 * Src1 + C0 + C1, reference=_reference),
        subdim=False,
        uops_sha={},
    )
    row = max(_SUB_OPCODE_FOR_NAME.values()) + 1
    assert row < 0x20, f"no free opcode-table row for {name}"
    _SUB_OPCODE_FOR_NAME[name] = row
    OPS.append(op)
    CUSTOM_DVE_SPECS[name] = op.spec
    return op


def _build_nc():
    import concourse.bass as bass
    import concourse.bacc as bacc
    import concourse.mybir as mybir
    import concourse.tile as tile
    from contextlib import ExitStack

    MINGRU_SCAN = _define_mingru_scan()

    f32 = mybir.dt.float32
    bf16 = mybir.dt.bfloat16
    AF = mybir.ActivationFunctionType
    OP = mybir.AluOpType

    nc = bacc.Bacc("TRN2", target_bir_lowering=False, debug=False)

    # x host layout: [128, BL, NTB, DC, TBLK] so each (b,tb) load is one
    # contiguous [128, DC*TBLK] DMA (each dma_start costs ~610ns of
    # sequencer DGE setup; fewer, bigger transfers compress the ramp).
    xt_ext = nc.declare_dram_parameter("xt", [128, BL, NTB, DC, TBLK], bf16,
                                       isOutput=False)
    wzt_ext = nc.declare_dram_parameter("wzt", [128, HC * DC * 128], bf16,
                                        isOutput=False)
    wht_ext = nc.declare_dram_parameter("wht", [128, HC * DC * 128], bf16,
                                        isOutput=False)
    bz_ext = nc.declare_dram_parameter("bz", [HC, 128, 1], f32, isOutput=False)
    nbh_ext = nc.declare_dram_parameter("negbh", [HC, 128, 1], f32, isOutput=False)
    itau_ext = nc.declare_dram_parameter("invtau", [BL, 1, T], bf16, isOutput=False)
    # per (b, pair, tb): [128, 2*TBLK] column-interleaved (A=even, B=odd)
    out_ext = nc.declare_dram_parameter(
        "out", [BL, PAIRS, NTB, 128, 2 * TBLK], bf16, isOutput=True
    )

    with tile.TileContext(nc) as tc, ExitStack() as ctx:
        singles = ctx.enter_context(tc.tile_pool(name="singles", bufs=1))
        x_pool = ctx.enter_context(tc.tile_pool(name="x", bufs=2))
        j_pool = ctx.enter_context(tc.tile_pool(name="j", bufs=2))
        psum = ctx.enter_context(tc.tile_pool(name="psum", bufs=2, space="PSUM"))
        u_pool = ctx.enter_context(tc.tile_pool(name="u", bufs=2))
        z_pool = ctx.enter_context(tc.tile_pool(name="z", bufs=2))
        h_pool = ctx.enter_context(tc.tile_pool(name="h", bufs=3))
        c_pool = ctx.enter_context(tc.tile_pool(name="carry", bufs=2))

        # Interleave the startup DMAs so the first z-matmul group (needs wz
        # + x dc0/dc1) is gated on ~0.75MB, not on everything.
        def xflat(b, tb, dc0, dcn):
            """Contiguous [128, dcn*TBLK] view of xt_ext[:, b, tb, dc0:dc0+dcn]."""
            a = xt_ext[:, b, tb, dc0, 0:1]
            return bass.AP(
                tensor=a.tensor, offset=a.offset,
                ap=[list(a.ap[0]), [1, dcn * TBLK]],
            )

        wz = singles.tile([128, HC * DC * 128], bf16, name="wz")
        nc.sync.dma_start(out=wz[:], in_=wzt_ext[:, :])
        x0 = x_pool.tile([128, DC * TBLK], bf16, tag="x")
        nc.sync.dma_start(out=x0[:, 0:2 * TBLK], in_=xflat(0, 0, 0, 2))
        wh = singles.tile([128, HC * DC * 128], bf16, name="wh")
        nc.sync.dma_start(out=wh[:], in_=wht_ext[:, :])
        nc.sync.dma_start(out=x0[:, 2 * TBLK:4 * TBLK], in_=xflat(0, 0, 2, 2))
        bz_col, nbh_col = [], []
        for hc in range(HC):
            c = singles.tile([128, 1], f32, name=f"bzc{hc}")
            nc.gpsimd.dma_start(out=c[:], in_=bz_ext[hc])
            bz_col.append(c)
            c = singles.tile([128, 1], f32, name=f"nbhc{hc}")
            nc.gpsimd.dma_start(out=c[:], in_=nbh_ext[hc])
            nbh_col.append(c)

        def wchunk(w, hc, dc):
            o = hc * DC * 128 + dc * 128
            return w[:, o:o + 128]

        # carry[(pair, stream)] = [128,1] fp32 AP with h' of the last
        # processed column for that hc chunk.
        carry = {}

        for b in range(BL):
            for tb in range(NTB):
                if b == 0 and tb == 0:
                    xs = x0
                else:
                    xs = x_pool.tile([128, DC * TBLK], bf16, tag="x")
                    nc.sync.dma_start(out=xs[:], in_=xflat(b, tb, 0, DC))
                jt = j_pool.tile([128, TBLK], bf16, tag="J")
                iv = itau_ext[b, 0, tb * TBLK:(tb + 1) * TBLK]
                iv_b = bass.AP(
                    tensor=iv.tensor, offset=iv.offset, ap=[[0, 128]] + list(iv.ap)
                )
                nc.gpsimd.dma_start(out=jt[:], in_=iv_b)

                for pair in range(PAIRS):
                    hcA, hcB = 2 * pair, 2 * pair + 1
                    last_group = (b == BL - 1 and tb == NTB - 1 and pair == PAIRS - 1)
                    # split the final group into 512-col sub-blocks so the
                    # post-matmul tail (STT->sigmoid->scan->DMA) is short
                    subs = ((0, 1), (1, 2)) if last_group else ((0, 2),)

                    zp = psum.tile([128, 2 * TBLK], f32, tag="zq")
                    qp_holder = [None]
                    u = u_pool.tile([128, 2 * TBLK], bf16, tag="u")
                    z = z_pool.tile([128, 2 * TBLK], bf16, tag="z")
                    h = h_pool.tile([128, 2 * TBLK], bf16, tag="h")

                    for h0, h1 in subs:
                        nh = h1 - h0
                        # z-preactivations: 2 psum banks per 512-col half
                        for s, hc in enumerate((hcA, hcB)):
                            for dc in range(DC):
                                for half in range(h0, h1):
                                    csl = slice(
                                        dc * TBLK + half * MMN,
                                        dc * TBLK + (half + 1) * MMN,
                                    )
                                    psl = slice(
                                        s * TBLK + half * MMN,
                                        s * TBLK + (half + 1) * MMN,
                                    )
                                    nc.tensor.matmul(
                                        zp[:, psl],
                                        lhsT=wchunk(wz, hc, dc),
                                        rhs=xs[:, csl],
                                        start=(dc == 0),
                                        stop=(dc == DC - 1),
                                    )

                        # u = (zpre + bz) * invtau, written column-interleaved
                        for s, hc in enumerate((hcA, hcB)):
                            u_int = bass.AP(
                                tensor=u[:].tensor,
                                offset=u[:].offset + 2 * h0 * MMN + s,
                                ap=[list(u[:].ap[0]), [2, nh * MMN]],
                            )
                            nc.vector.scalar_tensor_tensor(
                                u_int,
                                zp[:, s * TBLK + h0 * MMN:s * TBLK + h1 * MMN],
                                bz_col[hc][:],
                                jt[:, h0 * MMN:h1 * MMN],
                                op0=OP.add,
                                op1=OP.mult,
                            )

                        usl = slice(2 * h0 * MMN, 2 * h1 * MMN)
                        nc.scalar.activation(z[:, usl], u[:, usl], AF.Sigmoid)

                        # candidate q: the other psum banks
                        if qp_holder[0] is None:
                            qpt = psum.tile([128, 2 * TBLK], f32, tag="zq", name="qp")
                            qp_holder[0] = qpt
                        qp = qp_holder[0]
                        for s, hc in enumerate((hcA, hcB)):
                            for dc in range(DC):
                                for half in range(h0, h1):
                                    csl = slice(
                                        dc * TBLK + half * MMN,
                                        dc * TBLK + (half + 1) * MMN,
                                    )
                                    psl = slice(
                                        s * TBLK + half * MMN,
                                        s * TBLK + (half + 1) * MMN,
                                    )
                                    nc.tensor.matmul(
                                        qp[:, psl],
                                        lhsT=wchunk(wh, hc, dc),
                                        rhs=xs[:, csl],
                                        start=(dc == 0),
                                        stop=(dc == DC - 1),
                                    )

                        # fused interleaved scan: h' = (1-z) h'_prev + z q
                        qa = qp[:, h0 * MMN:h0 * MMN + nh * MMN]
                        q_pair = bass.AP(
                            tensor=qa.tensor, offset=qa.offset,
                            ap=list(qa.ap) + [[TBLK, 2]],
                        )
                        if tb == 0 and h0 == 0:
                            s0, s1 = nbh_col[hcA][:], nbh_col[hcB][:]
                        else:
                            s0, s1 = carry[(pair, 0)], carry[(pair, 1)]
                        nc.vector._custom_dve(
                            MINGRU_SCAN,
                            out=h[:, usl], in0=q_pair, in1=z[:, usl],
                            s0=s0, s1=s1,
                        )
                        if not (b == BL - 1 and tb == NTB - 1 and h1 == 2):
                            # custom-DVE scalar reads must be fp32: stage the
                            # last column pair through a small fp32 tile
                            ct = c_pool.tile([128, 2], f32, tag=f"c{pair}")
                            nc.scalar.copy(
                                ct[:], h[:, 2 * h1 * MMN - 2:2 * h1 * MMN]
                            )
                            carry[(pair, 0)] = ct[:, 0:1]
                            carry[(pair, 1)] = ct[:, 1:2]

                        nc.scalar.dma_start(
                            out=out_ext[b, pair, tb, :, usl], in_=h[:, usl]
                        )

    nc.compile()
    return nc


def _prep_inputs(x, motion_mag, Wz, bz, Wh, bh, motion_weight, motion_bias, alpha):
    bf = ml_dtypes.bfloat16
    x = np.asarray(x, dtype=np.float32)
    mm = np.asarray(motion_mag, dtype=np.float32)
    Wz = np.asarray(Wz, dtype=np.float32)
    Wh = np.asarray(Wh, dtype=np.float32)
    bz = np.asarray(bz, dtype=np.float32).reshape(HC, 128, 1)
    bh = np.asarray(bh, dtype=np.float32).reshape(HC, 128, 1)
    mw = float(np.asarray(motion_weight))
    mb = float(np.asarray(motion_bias))
    al = float(np.asarray(alpha))

    a_sp = float(np.log1p(np.exp(al)))  # softplus(alpha)
    sig = 1.0 / (1.0 + np.exp(-(mw * mm + mb)))
    invtau = (1.0 / (1.0 + a_sp * sig)).astype(bf)

    # lhsT chunk (hc, dc) = wzt[:, (hc*DC+dc)*128 : +128] = Wz.T[dc-block, hc-block]
    wzt = np.ascontiguousarray(
        Wz.T.reshape(DC, 128, HC, 128).transpose(1, 2, 0, 3).reshape(
            128, HC * DC * 128)).astype(bf)
    wht = np.ascontiguousarray(
        Wh.T.reshape(DC, 128, HC, 128).transpose(1, 2, 0, 3).reshape(
            128, HC * DC * 128)).astype(bf)

    in_maps = []
    for c in range(NCORES):
        xl = x[c * BL:(c + 1) * BL]  # [BL, T, D]
        xt = np.ascontiguousarray(
            xl.reshape(BL, NTB, TBLK, DC, 128).transpose(4, 0, 1, 3, 2)
        ).astype(bf)
        in_maps.append({
            "xt": xt,
            "wzt": wzt,
            "wht": wht,
            "bz": bz,
            "negbh": -bh,
            "invtau": np.ascontiguousarray(
                invtau[c * BL:(c + 1) * BL]).reshape(BL, 1, T),
        })
    return in_maps, bh


def _assemble(results, bh):
    outs = []
    for c in range(NCORES):
        o = np.asarray(results[c]["out"], dtype=np.float32)
        # [BL, PAIRS, NTB, 128, 2*TBLK] -> [BL, T, H]
        o = o.reshape(BL, PAIRS, NTB, 128, TBLK, 2)
        o = np.transpose(o, (0, 2, 4, 1, 5, 3)).reshape(BL, T, H)
        outs.append(o)
    full = np.ascontiguousarray(np.concatenate(outs, axis=0))
    bhf = bh.reshape(H)
    if np.any(bhf):
        full += bhf
    return full


def _run(inputs, trace=False):
    from concourse.bass_utils import run_bass_kernel_spmd

    if "nc" not in _CACHE:
        _CACHE["nc"] = _build_nc()
    nc = _CACHE["nc"]
    in_maps, bh = _prep_inputs(**inputs)
    res = run_bass_kernel_spmd(nc, in_maps, list(range(NCORES)), trace=trace)
    return _assemble(res.results, bh), res


def kernel(**inputs):
    out, _ = _run(inputs, trace=False)
    return out


# revision 50
# speedup vs baseline: 1.3745x; 1.3745x over previous
"""MinGRU cell kernel for Trainium2 (8 NeuronCores, data-parallel over batch).

Computes, for x:[B,T,D], motion_mag:[B,T]:
    tau = 1 + softplus(alpha) * sigmoid(mw*mm + mb)        (per b,t)
    z   = sigmoid((x @ Wz^T + bz) / tau)                   (B,T,H)
    q   = x @ Wh^T (+ bh)                                  (B,T,H)
    h_t = (1-z_t)*h_{t-1} + z_t*q_t   (scan over t, h_0=0)

Strategy (vs the stock-op baseline at ~182us):
  - bf16 matmuls and bf16 HBM I/O: halves DMA traffic and SBUF footprint;
    PE rate is identical to fp32r (1 col/cycle). End-to-end rel err ~4e-3
    (gate 2e-2).
  - The whole elementwise recurrence tail is ONE custom DVE instruction
    (MINGRU_SCAN_ANT, hand-written uOp program): it consumes the gate z
    (SBUF bf16) and the candidate q (PSUM fp32) directly and computes
    h = (1-z)*h_prev + z*q at 1 element/cycle by interleaving the two
    hc-chunk streams of an hc-pair (stock tensor_tensor_scan runs at 2
    cycles/element and needs two extra DVE passes + an extra ACT pass to
    form (1-z) and z*q). DVE busy drops ~146us -> ~75us.
  - bh is folded out via h' = h - bh: the scan runs on raw q with initial
    carry -bh, and bh is re-added on the host (bh==0 here, so it's free).
  - Per (b, tb, pair) group: 16 z-matmuls -> one 4-bank PSUM tile, 2
    STT-u (u=(zpre+bz)*invtau, written column-interleaved), 1 sigmoid
    over the interleaved pair, 16 q-matmuls -> second 4-bank PSUM tile,
    1 fused scan, 1 out-DMA. Two 4-bank PSUM tiles ping-pong.
"""

import sys

import numpy as np

if "/opt/trn_rl_repo" not in sys.path:
    sys.path.insert(0, "/opt/trn_rl_repo")

import ml_dtypes

B, T, D, H = 32, 2048, 512, 512
NCORES = 8
BL = B // NCORES            # batch per core = 4
TBLK = 1024                 # t-columns per group
NTB = T // TBLK             # 2 t-blocks per sample
MMN = 512                   # matmul free-dim (1 psum bank)
DC = D // 128               # 4 contraction chunks
HC = H // 128               # 4 h partition chunks
PAIRS = HC // 2             # 2 hc-pairs (interleaved scan streams)
BT = BL * T                 # 8192 columns per core

# z-path precision: the gate matmul tolerates fp8 (the candidate path does
# not). False = hybrid (d<256 fp8-DoubleRow, d>=256 scaled-bf16, measured
# rel err 1.32e-2); True = all-fp8 gate (measured 1.83e-2, gate 2e-2).
# The hybrid is faster in practice: the all-fp8 z-phase undershoots the
# DVE/consumer pipeline and stalls the PE at PSUM handoffs.
FULL_Z_FP8 = False
ZDC = 2 if FULL_Z_FP8 else 1   # dc-pairs of the z contraction done in fp8

_CACHE = {}

# --------------------------------------------------------------------------- #
# Custom DVE op: fused interleaved minGRU scan.
#
# Over a stream of 2*N elements interleaving two recurrences A (even k) and
# B (odd k):
#     h_k = (1 - z_k) * h_{k-2} + z_k * q_k     h_{-2}=s0[p], h_{-1}=s1[p]
#
# Datapath (8 stages, v3/TRN2):
#   inputs: lane0 = SRC_0 (q), lane1 = SRC_1 (z), lane2 = ONE_F32
#   stage 0: d = q * z           ; carry z (lane1), 1.0 (lane2)
#   stage 1: e = 1.0 - z         ; capture d -> lane3
#   stage 2: m = e * h_prev      ; h_prev via NEXT_ALU_OUT_A/B (stage-3 flop)
#   stage 3: h = m + d           ; a-flop (stream A) / b-flop (stream B)
#   stages 4-7: BYPASS chain to the write port.
#
# Each stream's running h lives in its own stage-3 flop, maintained by two
# alternating steady uOps (uOp transitions are zero-cost), which makes the
# recurrence immune to pipeline stalls: a flop holds its last value until
# the next element of the SAME stream rewrites it. Two 1-cycle
# non-consuming seed uOps preload the flops from CONST_0/CONST_1 (the
# [P,1] h-init carries). Measured: 2339ns per 2048-element instruction
# (1.14 cyc/elem); exact vs numpy in fp32.
# --------------------------------------------------------------------------- #


def _define_mingru_scan():
    from concourse.dve_ops import (
        OPS,
        _SUB_OPCODE_FOR_NAME,
        CUSTOM_DVE_SPECS,
        DveOp,
    )
    from concourse.dve_spec import C0, C1, Spec, Src0, Src1
    from concourse.dve_uop import (
        ENABLE,
        AluInp,
        AluOp,
        DelayInp,
        DveOpSpec,
        InpSel,
        OutPath,
        OutSel,
        Trigger,
        UopConfig,
    )

    name = "MINGRU_SCAN_ANT"
    if name in _SUB_OPCODE_FOR_NAME:
        return next(op for op in OPS if op.name == name)

    def _steady(stream_b, other_idx):
        u = UopConfig()
        u.enable_input(InpSel.SRC_0, 1)     # lane0 = q
        u.enable_input(InpSel.SRC_1, 2)     # lane1 = z
        u.enable_input(InpSel.ONE_F32, 3)   # lane2 = 1.0
        dp = u.datapath_config
        dp[0].enable_alu(AluOp.MULTIPLY, AluInp.PREV_DELAY_0, AluInp.PREV_DELAY_1)
        dp[0].pass_through_delay(1, 2)
        dp[1].enable_alu(AluOp.SUBTRACT, AluInp.PREV_DELAY_2, AluInp.PREV_DELAY_1)
        dp[1].enable_delay_from_src(DelayInp.PREV_ALU_OUT, 3)
        dp[2].enable_alu(
            AluOp.MULTIPLY,
            AluInp.PREV_ALU_OUT,
            AluInp.NEXT_ALU_OUT_B if stream_b else AluInp.NEXT_ALU_OUT_A,
        )
        dp[2].pass_through_delay(3)
        dp[3].enable_alu(AluOp.ADD, AluInp.PREV_ALU_OUT, AluInp.PREV_DELAY_3)
        if stream_b:
            dp[3].alu_out_b_enable = ENABLE
        else:
            dp[3].alu_out_a_enable = ENABLE
        for s in range(4, 8):
            dp[s].pass_through_alu()
        u.enable_output(OutSel.ALU_OUT, OutPath.WR0_LO)
        u.require_inp0 = ENABLE
        u.require_inp1 = ENABLE
        u.repeat_count = 1
        u.trigger = (Trigger.SRC_TENSOR_DONE, Trigger.COUNT, Trigger.NONE)
        u.next_uop = (0, other_idx, 0)
        return u

    def _seed(const, stream_b, next_idx):
        u = UopConfig()
        u.enable_input(const, 1)
        dp = u.datapath_config
        dp[0].enable_alu(AluOp.BYPASS, AluInp.PREV_DELAY_0, AluInp.PREV_DELAY_0)
        for s in range(1, 8):
            dp[s].pass_through_alu()
        if stream_b:
            dp[3].alu_out_b_enable = ENABLE
        else:
            dp[3].alu_out_a_enable = ENABLE
        u.repeat_count = 1
        u.trigger = (Trigger.COUNT, Trigger.NONE, Trigger.NONE)
        u.next_uop = (next_idx, 0, 0)
        return u

    def _reference(in0, in1, s0, s1, imm2):
        P = in0.shape[0]
        q = np.asarray(in0, np.float32).reshape(P, -1)
        z = np.asarray(in1, np.float32).reshape(P, -1)
        n2 = q.shape[1]
        h = np.empty((P, n2), np.float32)
        prev = [
            np.broadcast_to(np.asarray(s0, np.float32).reshape(-1), (P,)).copy(),
            np.broadcast_to(np.asarray(s1, np.float32).reshape(-1), (P,)).copy(),
        ]
        for k in range(n2):
            s = k & 1
            prev[s] = (1.0 - z[:, k]) * prev[s] + z[:, k] * q[:, k]
            h[:, k] = prev[s]
        return h

    class _HandWrittenDveOp(DveOp):
        def compile(self, ver):
            assert ver == "v3", f"{name} only authored for v3/TRN2, got {ver}"
            s = DveOpSpec(
                name=self.name,
                opcode=_SUB_OPCODE_FOR_NAME[self.name],
                uops=[
                    _seed(InpSel.CONST_0, False, 1),
                    _seed(InpSel.CONST_1, True, 2),
                    _steady(False, 3),
                    _steady(True, 2),
                ],
                rd1_en=True,
            )
            s.validate(ver)
            return s

    op = _HandWrittenDveOp(
        name,
        Spec(body=Src0 * Src1 + C0 + C1, reference=_reference),
        subdim=False,
        uops_sha={},
    )
    row = max(_SUB_OPCODE_FOR_NAME.values()) + 1
    assert row < 0x20, f"no free opcode-table row for {name}"
    _SUB_OPCODE_FOR_NAME[name] = row
    OPS.append(op)
    CUSTOM_DVE_SPECS[name] = op.spec
    return op


def _build_nc():
    import concourse.bass as bass
    import concourse.bacc as bacc
    import concourse.mybir as mybir
    import concourse.tile as tile
    from contextlib import ExitStack

    MINGRU_SCAN = _define_mingru_scan()

    f32 = mybir.dt.float32
    bf16 = mybir.dt.bfloat16
    AF = mybir.ActivationFunctionType
    OP = mybir.AluOpType

    nc = bacc.Bacc("TRN2", target_bir_lowering=False, debug=False)

    f8 = mybir.dt.float8e4

    # x host layout: [128, BL, NTB, DC, TBLK] so each (b,tb) load is one
    # contiguous [128, DC*TBLK] DMA (each dma_start costs ~610ns of
    # sequencer DGE setup; fewer, bigger transfers compress the ramp).
    xt_ext = nc.declare_dram_parameter("xt", [128, BL, NTB, DC, TBLK], bf16,
                                       isOutput=False)
    # z-path fp8 inputs: d in [0, ZDC*256) as fp8 x*16 for DoubleRow
    # matmuls; layout [p, b, tb, i, col] with d = i*128 + p.
    x8_ext = nc.declare_dram_parameter("x8", [128, BL, NTB, 2 * ZDC, TBLK], f8,
                                       isOutput=False)
    # Wz*2048 fp8: [p, hc*(ZDC*256) + i*128 + m]
    wz8_ext = nc.declare_dram_parameter("wz8", [128, HC * 2 * ZDC * 128], f8,
                                        isOutput=False)
    # Hybrid only: Wz*32768 bf16 for d in [256,512) (scale exact in bf16):
    # fp8 and bf16 z contributions then accumulate consistently (both
    # 2^15-scaled) in one PSUM group; the STT descales via invtau/2^15 and
    # bz*2^15.
    if not FULL_Z_FP8:
        wzt_ext = nc.declare_dram_parameter("wzt", [128, HC * 2 * 128], bf16,
                                            isOutput=False)
    wht_ext = nc.declare_dram_parameter("wht", [128, HC * DC * 128], bf16,
                                        isOutput=False)
    # columns: [bz per hc | -bh per hc]
    bias_ext = nc.declare_dram_parameter("bias", [128, 2 * HC], f32, isOutput=False)
    itau_ext = nc.declare_dram_parameter("invtau", [BL, 1, T], bf16, isOutput=False)
    # per (b, pair, tb): [128, 2*TBLK] column-interleaved (A=even, B=odd)
    out_ext = nc.declare_dram_parameter(
        "out", [BL, PAIRS, NTB, 128, 2 * TBLK], bf16, isOutput=True
    )

    with tile.TileContext(nc) as tc, ExitStack() as ctx:
        singles = ctx.enter_context(tc.tile_pool(name="singles", bufs=1))
        x_pool = ctx.enter_context(tc.tile_pool(name="x", bufs=2))
        j_pool = ctx.enter_context(tc.tile_pool(name="j", bufs=2))
        psum = ctx.enter_context(tc.tile_pool(name="psum", bufs=2, space="PSUM"))
        u_pool = ctx.enter_context(tc.tile_pool(name="u", bufs=2))
        z_pool = ctx.enter_context(tc.tile_pool(name="z", bufs=2))
        h_pool = ctx.enter_context(tc.tile_pool(name="h", bufs=3))
        c_pool = ctx.enter_context(tc.tile_pool(name="carry", bufs=2))

        # Interleave the startup DMAs so the first z-matmul group (needs wz
        # + x dc0/dc1) is gated on ~0.75MB, not on everything.
        def xflat(b, tb, dc0, dcn):
            """Contiguous [128, dcn*TBLK] view of xt_ext[:, b, tb, dc0:dc0+dcn]."""
            a = xt_ext[:, b, tb, dc0, 0:1]
            return bass.AP(
                tensor=a.tensor, offset=a.offset,
                ap=[list(a.ap[0]), [1, dcn * TBLK]],
            )

        # Per-pair weight tiles: the first z-matmuls gate only on wz_p[0]
        # (dep tracking is per-tile, so split DMAs into one tile don't help).
        # PE warm-up: ~20 matmuls on garbage SBUF data, no input deps, so
        # they issue the moment the PE sequencer starts (~7us) and hold the
        # p-state at 2.4GHz until the real data lands (~12us). Their PSUM
        # writes are reset by the first real start=True matmul.
        warm = singles.tile([128, MMN], bf16, name="warm")
        nc.gpsimd.memset(warm[:], 0.0)
        wpsum = psum.tile([128, 2 * TBLK], f32, tag="zq", name="wpsum")
        # 128-col warm matmuls: fine-grained so the warm stream ends close
        # to when the first real operands land (~12.5us); ~30 x ~150ns
        # (cold-clock) covers the window without delaying real work.
        for _ in range(30):
            nc.tensor.matmul(
                wpsum[:, 0:128], lhsT=warm[:, 0:128], rhs=warm[:, 0:128],
                start=True, stop=True,
            )

        def x8flat(b, tb):
            a = x8_ext[:, b, tb, 0, 0:1]
            return bass.AP(
                tensor=a.tensor, offset=a.offset,
                ap=[list(a.ap[0]), [1, 2 * ZDC * TBLK]],
            )

        PW = 2 * DC * 128        # bf16 wh chunk-cols per pair
        PZ = 2 * 2 * ZDC * 128   # fp8 z chunk-cols per pair
        PB = 2 * 2 * 128         # scaled-bf16 z chunk-cols per pair (hybrid)
        wz8_p, wzb_p, wh_p = [None] * PAIRS, [None] * PAIRS, [None] * PAIRS
        wz8_p[0] = singles.tile([128, PZ], f8, name="wz8p0")
        nc.sync.dma_start(out=wz8_p[0][:], in_=wz8_ext[:, 0:PZ])
        if not FULL_Z_FP8:
            wzb_p[0] = singles.tile([128, PB], bf16, name="wzbp0")
            nc.sync.dma_start(out=wzb_p[0][:], in_=wzt_ext[:, 0:PB])
        # First x block: chunks spread over engine queues so the transfers
        # run on multiple DMA rings in parallel (a single 1MB dma_start was
        # observed to take ~8us). The z-phase needs x8 (+ bf16 dc2/dc3 when
        # hybrid) first.
        x0 = x_pool.tile([128, DC * TBLK], bf16, tag="x")
        x08 = x_pool.tile([128, 2 * ZDC * TBLK], f8, tag="x8")
        nc.scalar.dma_start(out=x08[:], in_=x8flat(0, 0))
        nc.gpsimd.dma_start(out=x0[:, 2 * TBLK:3 * TBLK], in_=xflat(0, 0, 2, 1))
        nc.scalar.dma_start(out=x0[:, 3 * TBLK:4 * TBLK], in_=xflat(0, 0, 3, 1))
        nc.gpsimd.dma_start(out=x0[:, 0:TBLK], in_=xflat(0, 0, 0, 1))
        nc.scalar.dma_start(out=x0[:, TBLK:2 * TBLK], in_=xflat(0, 0, 1, 1))
        # weight DMAs in first-use order: the z-phase is fast (~4.3us per
        # (b,tb)) so pair-1 z weights and pair-0 q weights are needed early
        for pr in range(1, PAIRS):
            wz8_p[pr] = singles.tile([128, PZ], f8, name=f"wz8p{pr}")
            nc.sync.dma_start(out=wz8_p[pr][:], in_=wz8_ext[:, pr * PZ:(pr + 1) * PZ])
            if not FULL_Z_FP8:
                wzb_p[pr] = singles.tile([128, PB], bf16, name=f"wzbp{pr}")
                nc.sync.dma_start(
                    out=wzb_p[pr][:], in_=wzt_ext[:, pr * PB:(pr + 1) * PB])
        wh_p[0] = singles.tile([128, PW], bf16, name="whp0")
        nc.sync.dma_start(out=wh_p[0][:], in_=wht_ext[:, 0:PW])
        for pr in range(1, PAIRS):
            wh_p[pr] = singles.tile([128, PW], bf16, name=f"whp{pr}")
            nc.sync.dma_start(out=wh_p[pr][:], in_=wht_ext[:, pr * PW:(pr + 1) * PW])
        bias_t = singles.tile([128, 2 * HC], f32, name="bias_t")
        nc.gpsimd.dma_start(out=bias_t[:], in_=bias_ext[:, :])
        bz_col = [bias_t[:, hc:hc + 1] for hc in range(HC)]
        nbh_col = [bias_t[:, HC + hc:HC + hc + 1] for hc in range(HC)]

        def wchunk(wp, hc, dc):
            o = (hc % 2) * DC * 128 + dc * 128
            return wp[hc // 2][:, o:o + 128]

        # carry[(pair, stream)] = [128,1] fp32 AP with h' of the last
        # processed column for that hc chunk.
        carry = {}

        for b in range(BL):
            for tb in range(NTB):
                if b == 0 and tb == 0:
                    xs, xs8 = x0, x08
                else:
                    # split across queues to parallelize the transfers
                    xs = x_pool.tile([128, DC * TBLK], bf16, tag="x")
                    nc.sync.dma_start(
                        out=xs[:, 0:2 * TBLK], in_=xflat(b, tb, 0, 2)
                    )
                    nc.gpsimd.dma_start(
                        out=xs[:, 2 * TBLK:], in_=xflat(b, tb, 2, 2)
                    )
                    xs8 = x_pool.tile([128, 2 * ZDC * TBLK], f8, tag="x8")
                    nc.sync.dma_start(out=xs8[:], in_=x8flat(b, tb))
                jt = j_pool.tile([128, TBLK], bf16, tag="J")
                iv = itau_ext[b, 0, tb * TBLK:(tb + 1) * TBLK]
                iv_b = bass.AP(
                    tensor=iv.tensor, offset=iv.offset, ap=[[0, 128]] + list(iv.ap)
                )
                nc.gpsimd.dma_start(out=jt[:], in_=iv_b)

                # Emit all z-paths first, then all q-paths: every PSUM
                # buffer handoff then has a full matmul-group (~3.4us) of
                # slack, so the PE never stalls at group boundaries (stalls
                # also drop it out of its 2.4GHz p-state).
                zs = [None] * PAIRS
                for pair in range(PAIRS):
                    hcA, hcB = 2 * pair, 2 * pair + 1

                    # z-preactivations for both streams: 4 psum banks.
                    # Per 512-col half: one fp8 DoubleRow matmul covers
                    # d in [0,256) (K=256 per instruction, 2x rate), then
                    # two scaled-bf16 matmuls cover d in [256,512).
                    zp = psum.tile([128, 2 * TBLK], f32, tag="zq")
                    for s, hc in enumerate((hcA, hcB)):
                        # fp8 DoubleRow chunks (K=256 each), weight-major so
                        # each lhsT loads once per 2 matmuls
                        for dp in range(ZDC):
                            o8 = (hc % 2) * ZDC * 256 + dp * 256
                            w8c = wz8_p[hc // 2][:, o8:o8 + 128]
                            w8_ap = bass.AP(
                                tensor=w8c.tensor, offset=w8c.offset,
                                ap=[list(w8c.ap[0]), [128, 2], [1, 128]],
                            )
                            for half in range(TBLK // MMN):
                                psl = slice(
                                    s * TBLK + half * MMN,
                                    s * TBLK + (half + 1) * MMN,
                                )
                                xo = 2 * dp * TBLK + half * MMN
                                x8c = xs8[:, xo:xo + MMN]
                                x8_ap = bass.AP(
                                    tensor=x8c.tensor, offset=x8c.offset,
                                    ap=[list(x8c.ap[0]), [TBLK, 2], [1, MMN]],
                                )
                                nc.tensor.matmul(
                                    zp[:, psl], lhsT=w8_ap, rhs=x8_ap,
                                    start=(dp == 0),
                                    stop=(FULL_Z_FP8 and dp == ZDC - 1),
                                    perf_mode=mybir.MatmulPerfMode.DoubleRow,
                                )
                        if not FULL_Z_FP8:
                            for dc in (2, 3):
                                wbc = wzb_p[hc // 2][
                                    :, (hc % 2) * 256 + (dc - 2) * 128:
                                    (hc % 2) * 256 + (dc - 1) * 128
                                ]
                                for half in range(TBLK // MMN):
                                    psl = slice(
                                        s * TBLK + half * MMN,
                                        s * TBLK + (half + 1) * MMN,
                                    )
                                    csl = slice(
                                        dc * TBLK + half * MMN,
                                        dc * TBLK + (half + 1) * MMN,
                                    )
                                    nc.tensor.matmul(
                                        zp[:, psl],
                                        lhsT=wbc,
                                        rhs=xs[:, csl],
                                        start=False,
                                        stop=(dc == 3),
                                    )

                    # u = (zpre + bz) * invtau, written column-interleaved
                    u = u_pool.tile([128, 2 * TBLK], bf16, tag="u")
                    for s, hc in enumerate((hcA, hcB)):
                        u_int = bass.AP(
                            tensor=u[:].tensor,
                            offset=u[:].offset + s,
                            ap=[list(u[:].ap[0]), [2, TBLK]],
                        )
                        nc.vector.scalar_tensor_tensor(
                            u_int,
                            zp[:, s * TBLK:(s + 1) * TBLK],
                            bz_col[hc],
                            jt[:],
                            op0=OP.add,
                            op1=OP.mult,
                        )

                    z = z_pool.tile([128, 2 * TBLK], bf16, tag="z")
                    nc.scalar.activation(z[:], u[:], AF.Sigmoid)
                    zs[pair] = z

                for pair in range(PAIRS):
                    hcA, hcB = 2 * pair, 2 * pair + 1

                    # candidate q for both streams; dc order (2,3,0,1) so the
                    # first (b,tb) can start before its x dc0/dc1 chunks land
                    qp = psum.tile([128, 2 * TBLK], f32, tag="zq")
                    for s, hc in enumerate((hcA, hcB)):
                        for idc, dc in enumerate((2, 3, 0, 1)):
                            for half in range(TBLK // MMN):
                                csl = slice(
                                    dc * TBLK + half * MMN,
                                    dc * TBLK + (half + 1) * MMN,
                                )
                                psl = slice(
                                    s * TBLK + half * MMN,
                                    s * TBLK + (half + 1) * MMN,
                                )
                                nc.tensor.matmul(
                                    qp[:, psl],
                                    lhsT=wchunk(wh_p, hc, dc),
                                    rhs=xs[:, csl],
                                    start=(idc == 0),
                                    stop=(idc == DC - 1),
                                )

                    # fused interleaved scan: h' = (1-z) h'_prev + z q
                    qa = qp[:, 0:TBLK]
                    q_pair = bass.AP(
                        tensor=qa.tensor, offset=qa.offset,
                        ap=list(qa.ap) + [[TBLK, 2]],
                    )
                    h = h_pool.tile([128, 2 * TBLK], bf16, tag="h")
                    if tb == 0:
                        s0, s1 = nbh_col[hcA], nbh_col[hcB]
                    else:
                        s0, s1 = carry[(pair, 0)], carry[(pair, 1)]
                    last_group = (
                        b == BL - 1 and tb == NTB - 1 and pair == PAIRS - 1
                    )
                    if not last_group:
                        nc.vector._custom_dve(
                            MINGRU_SCAN,
                            out=h[:], in0=q_pair, in1=zs[pair][:], s0=s0, s1=s1,
                        )
                        if tb < NTB - 1:
                            # custom-DVE scalar reads must be fp32: stage the
                            # last column pair through a small fp32 tile
                            ct = c_pool.tile([128, 2], f32, tag=f"c{pair}")
                            nc.scalar.copy(ct[:], h[:, 2 * TBLK - 2:2 * TBLK])
                            carry[(pair, 0)] = ct[:, 0:1]
                            carry[(pair, 1)] = ct[:, 1:2]
                        nc.scalar.dma_start(out=out_ext[b, pair, tb], in_=h[:])
                    else:
                        # final group: scan + DMA in halves so the out-DMA of
                        # the first half overlaps the second half's scan
                        MH = TBLK // 2
                        qh0 = qp[:, 0:MH]
                        nc.vector._custom_dve(
                            MINGRU_SCAN,
                            out=h[:, 0:TBLK],
                            in0=bass.AP(tensor=qh0.tensor, offset=qh0.offset,
                                        ap=list(qh0.ap) + [[TBLK, 2]]),
                            in1=zs[pair][:, 0:TBLK], s0=s0, s1=s1,
                        )
                        ct = c_pool.tile([128, 2], f32, tag=f"c{pair}")
                        nc.scalar.copy(ct[:], h[:, TBLK - 2:TBLK])
                        nc.scalar.dma_start(
                            out=out_ext[b, pair, tb, :, 0:TBLK], in_=h[:, 0:TBLK]
                        )
                        qh1 = qp[:, MH:2 * MH]
                        nc.vector._custom_dve(
                            MINGRU_SCAN,
                            out=h[:, TBLK:],
                            in0=bass.AP(tensor=qh1.tensor, offset=qh1.offset,
                                        ap=list(qh1.ap) + [[TBLK, 2]]),
                            in1=zs[pair][:, TBLK:],
                            s0=ct[:, 0:1], s1=ct[:, 1:2],
                        )
                        nc.scalar.dma_start(
                            out=out_ext[b, pair, tb, :, TBLK:], in_=h[:, TBLK:]
                        )

    nc.compile()
    return nc


def _prep_inputs(x, motion_mag, Wz, bz, Wh, bh, motion_weight, motion_bias, alpha):
    bf = ml_dtypes.bfloat16
    x = np.asarray(x, dtype=np.float32)
    mm = np.asarray(motion_mag, dtype=np.float32)
    Wz = np.asarray(Wz, dtype=np.float32)
    Wh = np.asarray(Wh, dtype=np.float32)
    bz = np.asarray(bz, dtype=np.float32)
    bh = np.asarray(bh, dtype=np.float32)
    # [128, 2*HC]: bz columns then -bh columns, per hc chunk
    bias = np.concatenate(
        [bz.reshape(HC, 128).T, -bh.reshape(HC, 128).T], axis=1
    ).astype(np.float32)
    mw = float(np.asarray(motion_weight))
    mb = float(np.asarray(motion_bias))
    al = float(np.asarray(alpha))

    f8 = ml_dtypes.float8_e4m3fn
    XS, WS = 16.0, 2048.0            # fp8 scales; combined 2^15
    SC = XS * WS

    a_sp = float(np.log1p(np.exp(al)))  # softplus(alpha)
    sig = 1.0 / (1.0 + np.exp(-(mw * mm + mb)))
    # z-PSUM is 2^15-scaled; descale via invtau, re-scale bz to match
    invtau = ((1.0 / (1.0 + a_sp * sig)) / SC).astype(bf)
    bias[:, 0:HC] *= SC

    WzT = Wz.T  # [D, H]
    D8 = ZDC * 256  # leading contraction depth done in fp8 on the z-path
    # fp8 z-weights, d in [0, D8): [p, hc*(2*ZDC*128) + i*128 + m]
    wz8 = np.ascontiguousarray(
        WzT[0:D8].reshape(2 * ZDC, 128, HC, 128).transpose(1, 2, 0, 3).reshape(
            128, HC * 2 * ZDC * 128))
    wz8 = np.clip(wz8 * WS, -200, 200).astype(f8)
    wht = np.ascontiguousarray(
        Wh.T.reshape(DC, 128, HC, 128).transpose(1, 2, 0, 3).reshape(
            128, HC * DC * 128)).astype(bf)

    in_maps = []
    for c in range(NCORES):
        xl = x[c * BL:(c + 1) * BL]  # [BL, T, D]
        xt = np.ascontiguousarray(
            xl.reshape(BL, NTB, TBLK, DC, 128).transpose(4, 0, 1, 3, 2)
        ).astype(bf)
        x8 = np.ascontiguousarray(
            xl[..., 0:D8].reshape(BL, NTB, TBLK, 2 * ZDC, 128)
            .transpose(4, 0, 1, 3, 2)
        )
        x8 = np.clip(x8 * XS, -200, 200).astype(f8)
        m = {
            "xt": xt,
            "x8": x8,
            "wz8": wz8,
            "wht": wht,
            "bias": bias,
            "invtau": np.ascontiguousarray(
                invtau[c * BL:(c + 1) * BL]).reshape(BL, 1, T),
        }
        if not FULL_Z_FP8:
            # scaled-bf16 z-weights, d in [256,512)
            m["wzt"] = np.ascontiguousarray(
                WzT[256:512].reshape(2, 128, HC, 128).transpose(1, 2, 0, 3)
                .reshape(128, HC * 2 * 128) * SC).astype(bf)
        in_maps.append(m)
    return in_maps, bh


def _assemble(results, bh):
    outs = []
    for c in range(NCORES):
        o = np.asarray(results[c]["out"], dtype=np.float32)
        # [BL, PAIRS, NTB, 128, 2*TBLK] -> [BL, T, H]
        o = o.reshape(BL, PAIRS, NTB, 128, TBLK, 2)
        o = np.transpose(o, (0, 2, 4, 1, 5, 3)).reshape(BL, T, H)
        outs.append(o)
    full = np.ascontiguousarray(np.concatenate(outs, axis=0))
    bhf = bh.reshape(H)
    if np.any(bhf):
        full += bhf
    return full


def _run(inputs, trace=False):
    from concourse.bass_utils import run_bass_kernel_spmd

    if "nc" not in _CACHE:
        _CACHE["nc"] = _build_nc()
    nc = _CACHE["nc"]
    in_maps, bh = _prep_inputs(**inputs)
    res = run_bass_kernel_spmd(nc, in_maps, list(range(NCORES)), trace=trace)
    return _assemble(res.results, bh), res


def kernel(**inputs):
    out, _ = _run(inputs, trace=False)
    return out


# revision 52
# speedup vs baseline: 1.3747x; 1.0002x over previous
"""MinGRU cell kernel for Trainium2 (8 NeuronCores, data-parallel over batch).

Computes, for x:[B,T,D], motion_mag:[B,T]:
    tau = 1 + softplus(alpha) * sigmoid(mw*mm + mb)        (per b,t)
    z   = sigmoid((x @ Wz^T + bz) / tau)                   (B,T,H)
    q   = x @ Wh^T (+ bh)                                  (B,T,H)
    h_t = (1-z_t)*h_{t-1} + z_t*q_t   (scan over t, h_0=0)

Strategy (~123us vs the stock-op fp32 baseline at ~182us):
  - bf16 matmuls and bf16 HBM I/O: halves DMA traffic and SBUF footprint;
    PE rate is identical to fp32r (1 col/cycle).
  - Gate (z) matmul in hybrid precision: the first half of the contraction
    runs as fp8 DoubleRow matmuls (K=256/instruction, 2x PE rate), the
    second half as 2^15-scaled bf16 (scale exact in bf16) so both halves
    accumulate consistently in one PSUM group; the STT descales via
    invtau/2^15. The candidate (q) path stays bf16 (its errors pass
    straight to the output). End-to-end rel err 1.32e-2 (gate 2e-2).
  - The whole elementwise recurrence tail is ONE custom DVE instruction
    (MINGRU_SCAN_ANT, hand-written uOp program): it consumes the gate z
    (SBUF bf16) and the candidate q (PSUM fp32) directly and computes
    h = (1-z)*h_prev + z*q at 1 element/cycle by interleaving the two
    hc-chunk streams of an hc-pair (stock tensor_tensor_scan runs at 2
    cycles/element and needs two extra DVE passes + an extra ACT pass to
    form (1-z) and z*q). DVE busy drops ~146us -> ~80us.
  - bh is folded out via h' = h - bh: the scan runs on raw q with initial
    carry -bh, and bh is re-added on the host (bh==0 here, so it's free).
  - Per (b,tb): all z-phases (matmuls + STT-u + sigmoid) for both hc-pairs
    are emitted before all q-phases (matmuls + scan + out-DMA), so every
    4-bank PSUM ping-pong handoff has a full matmul-group of slack and the
    PE never stalls (stalls also drop its 2.4GHz p-state).
  - Ramp: ~30 dependency-free warm-up matmuls on garbage SBUF hold the PE
    p-state while the first operands stream in; initial DMAs are split
    across the SP/Activation/GpSimd queues in first-use order.
  - Tail: the final group's scan + out-DMA run in two halves so the DMA of
    the first half overlaps the second half's scan.
"""

import sys

import numpy as np

if "/opt/trn_rl_repo" not in sys.path:
    sys.path.insert(0, "/opt/trn_rl_repo")

import ml_dtypes

B, T, D, H = 32, 2048, 512, 512
NCORES = 8
BL = B // NCORES            # batch per core = 4
TBLK = 1024                 # t-columns per group
NTB = T // TBLK             # 2 t-blocks per sample
MMN = 512                   # matmul free-dim (1 psum bank)
DC = D // 128               # 4 contraction chunks
HC = H // 128               # 4 h partition chunks
PAIRS = HC // 2             # 2 hc-pairs (interleaved scan streams)
BT = BL * T                 # 8192 columns per core

# z-path precision: the gate matmul tolerates fp8 (the candidate path does
# not). False = hybrid (d<256 fp8-DoubleRow, d>=256 scaled-bf16, measured
# rel err 1.32e-2); True = all-fp8 gate (measured 1.83e-2, gate 2e-2).
# The hybrid is faster in practice: the all-fp8 z-phase undershoots the
# DVE/consumer pipeline and stalls the PE at PSUM handoffs.
FULL_Z_FP8 = False
ZDC = 2 if FULL_Z_FP8 else 1   # dc-pairs of the z contraction done in fp8

_CACHE = {}

# --------------------------------------------------------------------------- #
# Custom DVE op: fused interleaved minGRU scan.
#
# Over a stream of 2*N elements interleaving two recurrences A (even k) and
# B (odd k):
#     h_k = (1 - z_k) * h_{k-2} + z_k * q_k     h_{-2}=s0[p], h_{-1}=s1[p]
#
# Datapath (8 stages, v3/TRN2):
#   inputs: lane0 = SRC_0 (q), lane1 = SRC_1 (z), lane2 = ONE_F32
#   stage 0: d = q * z           ; carry z (lane1), 1.0 (lane2)
#   stage 1: e = 1.0 - z         ; capture d -> lane3
#   stage 2: m = e * h_prev      ; h_prev via NEXT_ALU_OUT_A/B (stage-3 flop)
#   stage 3: h = m + d           ; a-flop (stream A) / b-flop (stream B)
#   stages 4-7: BYPASS chain to the write port.
#
# Each stream's running h lives in its own stage-3 flop, maintained by two
# alternating steady uOps (uOp transitions are zero-cost), which makes the
# recurrence immune to pipeline stalls: a flop holds its last value until
# the next element of the SAME stream rewrites it. Two 1-cycle
# non-consuming seed uOps preload the flops from CONST_0/CONST_1 (the
# [P,1] h-init carries). Measured: 2339ns per 2048-element instruction
# (1.14 cyc/elem); exact vs numpy in fp32.
# --------------------------------------------------------------------------- #


def _define_mingru_scan():
    from concourse.dve_ops import (
        OPS,
        _SUB_OPCODE_FOR_NAME,
        CUSTOM_DVE_SPECS,
        DveOp,
    )
    from concourse.dve_spec import C0, C1, Spec, Src0, Src1
    from concourse.dve_uop import (
        ENABLE,
        AluInp,
        AluOp,
        DelayInp,
        DveOpSpec,
        InpSel,
        OutPath,
        OutSel,
        Trigger,
        UopConfig,
    )

    name = "MINGRU_SCAN_ANT"
    if name in _SUB_OPCODE_FOR_NAME:
        return next(op for op in OPS if op.name == name)

    def _steady(stream_b, other_idx):
        u = UopConfig()
        u.enable_input(InpSel.SRC_0, 1)     # lane0 = q
        u.enable_input(InpSel.SRC_1, 2)     # lane1 = z
        u.enable_input(InpSel.ONE_F32, 3)   # lane2 = 1.0
        dp = u.datapath_config
        dp[0].enable_alu(AluOp.MULTIPLY, AluInp.PREV_DELAY_0, AluInp.PREV_DELAY_1)
        dp[0].pass_through_delay(1, 2)
        dp[1].enable_alu(AluOp.SUBTRACT, AluInp.PREV_DELAY_2, AluInp.PREV_DELAY_1)
        dp[1].enable_delay_from_src(DelayInp.PREV_ALU_OUT, 3)
        dp[2].enable_alu(
            AluOp.MULTIPLY,
            AluInp.PREV_ALU_OUT,
            AluInp.NEXT_ALU_OUT_B if stream_b else AluInp.NEXT_ALU_OUT_A,
        )
        dp[2].pass_through_delay(3)
        dp[3].enable_alu(AluOp.ADD, AluInp.PREV_ALU_OUT, AluInp.PREV_DELAY_3)
        if stream_b:
            dp[3].alu_out_b_enable = ENABLE
        else:
            dp[3].alu_out_a_enable = ENABLE
        for s in range(4, 8):
            dp[s].pass_through_alu()
        u.enable_output(OutSel.ALU_OUT, OutPath.WR0_LO)
        u.require_inp0 = ENABLE
        u.require_inp1 = ENABLE
        u.repeat_count = 1
        u.trigger = (Trigger.SRC_TENSOR_DONE, Trigger.COUNT, Trigger.NONE)
        u.next_uop = (0, other_idx, 0)
        return u

    def _seed(const, stream_b, next_idx):
        u = UopConfig()
        u.enable_input(const, 1)
        dp = u.datapath_config
        dp[0].enable_alu(AluOp.BYPASS, AluInp.PREV_DELAY_0, AluInp.PREV_DELAY_0)
        for s in range(1, 8):
            dp[s].pass_through_alu()
        if stream_b:
            dp[3].alu_out_b_enable = ENABLE
        else:
            dp[3].alu_out_a_enable = ENABLE
        u.repeat_count = 1
        u.trigger = (Trigger.COUNT, Trigger.NONE, Trigger.NONE)
        u.next_uop = (next_idx, 0, 0)
        return u

    def _reference(in0, in1, s0, s1, imm2):
        P = in0.shape[0]
        q = np.asarray(in0, np.float32).reshape(P, -1)
        z = np.asarray(in1, np.float32).reshape(P, -1)
        n2 = q.shape[1]
        h = np.empty((P, n2), np.float32)
        prev = [
            np.broadcast_to(np.asarray(s0, np.float32).reshape(-1), (P,)).copy(),
            np.broadcast_to(np.asarray(s1, np.float32).reshape(-1), (P,)).copy(),
        ]
        for k in range(n2):
            s = k & 1
            prev[s] = (1.0 - z[:, k]) * prev[s] + z[:, k] * q[:, k]
            h[:, k] = prev[s]
        return h

    class _HandWrittenDveOp(DveOp):
        def compile(self, ver):
            assert ver == "v3", f"{name} only authored for v3/TRN2, got {ver}"
            s = DveOpSpec(
                name=self.name,
                opcode=_SUB_OPCODE_FOR_NAME[self.name],
                uops=[
                    _seed(InpSel.CONST_0, False, 1),
                    _seed(InpSel.CONST_1, True, 2),
                    _steady(False, 3),
                    _steady(True, 2),
                ],
                rd1_en=True,
            )
            s.validate(ver)
            return s

    op = _HandWrittenDveOp(
        name,
        Spec(body=Src0 * Src1 + C0 + C1, reference=_reference),
        subdim=False,
        uops_sha={},
    )
    row = max(_SUB_OPCODE_FOR_NAME.values()) + 1
    assert row < 0x20, f"no free opcode-table row for {name}"
    _SUB_OPCODE_FOR_NAME[name] = row
    OPS.append(op)
    CUSTOM_DVE_SPECS[name] = op.spec
    return op


def _build_nc():
    import concourse.bass as bass
    import concourse.bacc as bacc
    import concourse.mybir as mybir
    import concourse.tile as tile
    from contextlib import ExitStack

    MINGRU_SCAN = _define_mingru_scan()

    f32 = mybir.dt.float32
    bf16 = mybir.dt.bfloat16
    AF = mybir.ActivationFunctionType
    OP = mybir.AluOpType

    nc = bacc.Bacc("TRN2", target_bir_lowering=False, debug=False)

    f8 = mybir.dt.float8e4

    # x host layout: [128, BL, NTB, DC, TBLK] so each (b,tb) load is one
    # contiguous [128, DC*TBLK] DMA (each dma_start costs ~610ns of
    # sequencer DGE setup; fewer, bigger transfers compress the ramp).
    xt_ext = nc.declare_dram_parameter("xt", [128, BL, NTB, DC, TBLK], bf16,
                                       isOutput=False)
    # z-path fp8 inputs: d in [0, ZDC*256) as fp8 x*16 for DoubleRow
    # matmuls; layout [p, b, tb, i, col] with d = i*128 + p.
    x8_ext = nc.declare_dram_parameter("x8", [128, BL, NTB, 2 * ZDC, TBLK], f8,
                                       isOutput=False)
    # Wz*2048 fp8: [p, hc*(ZDC*256) + i*128 + m]
    wz8_ext = nc.declare_dram_parameter("wz8", [128, HC * 2 * ZDC * 128], f8,
                                        isOutput=False)
    # Hybrid only: Wz*32768 bf16 for d in [256,512) (scale exact in bf16):
    # fp8 and bf16 z contributions then accumulate consistently (both
    # 2^15-scaled) in one PSUM group; the STT descales via invtau/2^15 and
    # bz*2^15.
    if not FULL_Z_FP8:
        wzt_ext = nc.declare_dram_parameter("wzt", [128, HC * 2 * 128], bf16,
                                            isOutput=False)
    wht_ext = nc.declare_dram_parameter("wht", [128, HC * DC * 128], bf16,
                                        isOutput=False)
    # columns: [bz per hc | -bh per hc]
    bias_ext = nc.declare_dram_parameter("bias", [128, 2 * HC], f32, isOutput=False)
    itau_ext = nc.declare_dram_parameter("invtau", [BL, 1, T], bf16, isOutput=False)
    # per (b, pair, tb): [128, 2*TBLK] column-interleaved (A=even, B=odd)
    out_ext = nc.declare_dram_parameter(
        "out", [BL, PAIRS, NTB, 128, 2 * TBLK], bf16, isOutput=True
    )

    with tile.TileContext(nc) as tc, ExitStack() as ctx:
        singles = ctx.enter_context(tc.tile_pool(name="singles", bufs=1))
        x_pool = ctx.enter_context(tc.tile_pool(name="x", bufs=2))
        j_pool = ctx.enter_context(tc.tile_pool(name="j", bufs=2))
        psum = ctx.enter_context(tc.tile_pool(name="psum", bufs=2, space="PSUM"))
        u_pool = ctx.enter_context(tc.tile_pool(name="u", bufs=2))
        z_pool = ctx.enter_context(tc.tile_pool(name="z", bufs=2))
        h_pool = ctx.enter_context(tc.tile_pool(name="h", bufs=3))
        c_pool = ctx.enter_context(tc.tile_pool(name="carry", bufs=2))

        # Interleave the startup DMAs so the first z-matmul group (needs wz
        # + x dc0/dc1) is gated on ~0.75MB, not on everything.
        def xflat(b, tb, dc0, dcn):
            """Contiguous [128, dcn*TBLK] view of xt_ext[:, b, tb, dc0:dc0+dcn]."""
            a = xt_ext[:, b, tb, dc0, 0:1]
            return bass.AP(
                tensor=a.tensor, offset=a.offset,
                ap=[list(a.ap[0]), [1, dcn * TBLK]],
            )

        # Per-pair weight tiles: the first z-matmuls gate only on wz_p[0]
        # (dep tracking is per-tile, so split DMAs into one tile don't help).
        # PE warm-up: ~20 matmuls on garbage SBUF data, no input deps, so
        # they issue the moment the PE sequencer starts (~7us) and hold the
        # p-state at 2.4GHz until the real data lands (~12us). Their PSUM
        # writes are reset by the first real start=True matmul.
        warm = singles.tile([128, MMN], bf16, name="warm")
        nc.gpsimd.memset(warm[:], 0.0)
        wpsum = psum.tile([128, 2 * TBLK], f32, tag="zq", name="wpsum")
        # 128-col warm matmuls: fine-grained so the warm stream ends close
        # to when the first real operands land (~12.5us); ~30 x ~150ns
        # (cold-clock) covers the window without delaying real work.
        for _ in range(30):
            nc.tensor.matmul(
                wpsum[:, 0:128], lhsT=warm[:, 0:128], rhs=warm[:, 0:128],
                start=True, stop=True,
            )

        def x8flat(b, tb):
            a = x8_ext[:, b, tb, 0, 0:1]
            return bass.AP(
                tensor=a.tensor, offset=a.offset,
                ap=[list(a.ap[0]), [1, 2 * ZDC * TBLK]],
            )

        PW = 2 * DC * 128        # bf16 wh chunk-cols per pair
        PZ = 2 * 2 * ZDC * 128   # fp8 z chunk-cols per pair
        PB = 2 * 2 * 128         # scaled-bf16 z chunk-cols per pair (hybrid)
        wz8_p, wzb_p, wh_p = [None] * PAIRS, [None] * PAIRS, [None] * PAIRS
        wz8_p[0] = singles.tile([128, PZ], f8, name="wz8p0")
        nc.sync.dma_start(out=wz8_p[0][:], in_=wz8_ext[:, 0:PZ])
        if not FULL_Z_FP8:
            wzb_p[0] = singles.tile([128, PB], bf16, name="wzbp0")
            nc.sync.dma_start(out=wzb_p[0][:], in_=wzt_ext[:, 0:PB])
        # First x block: chunks spread over engine queues so the transfers
        # run on multiple DMA rings in parallel (a single 1MB dma_start was
        # observed to take ~8us). The z-phase needs x8 (+ bf16 dc2/dc3 when
        # hybrid) first.
        x0 = x_pool.tile([128, DC * TBLK], bf16, tag="x")
        x08 = x_pool.tile([128, 2 * ZDC * TBLK], f8, tag="x8")
        nc.scalar.dma_start(out=x08[:], in_=x8flat(0, 0))
        nc.gpsimd.dma_start(out=x0[:, 2 * TBLK:3 * TBLK], in_=xflat(0, 0, 2, 1))
        nc.scalar.dma_start(out=x0[:, 3 * TBLK:4 * TBLK], in_=xflat(0, 0, 3, 1))
        nc.gpsimd.dma_start(out=x0[:, 0:TBLK], in_=xflat(0, 0, 0, 1))
        nc.scalar.dma_start(out=x0[:, TBLK:2 * TBLK], in_=xflat(0, 0, 1, 1))
        # (z-phase consumes x08 + dc2/dc3 first; q-phase dc order is
        # rotated (2,3,0,1) so dc0/dc1 have extra slack)
        # weight DMAs in first-use order: the z-phase is fast (~4.3us per
        # (b,tb)) so pair-1 z weights and pair-0 q weights are needed early
        for pr in range(1, PAIRS):
            wz8_p[pr] = singles.tile([128, PZ], f8, name=f"wz8p{pr}")
            nc.sync.dma_start(out=wz8_p[pr][:], in_=wz8_ext[:, pr * PZ:(pr + 1) * PZ])
            if not FULL_Z_FP8:
                wzb_p[pr] = singles.tile([128, PB], bf16, name=f"wzbp{pr}")
                nc.sync.dma_start(
                    out=wzb_p[pr][:], in_=wzt_ext[:, pr * PB:(pr + 1) * PB])
        wh_p[0] = singles.tile([128, PW], bf16, name="whp0")
        nc.sync.dma_start(out=wh_p[0][:], in_=wht_ext[:, 0:PW])
        for pr in range(1, PAIRS):
            wh_p[pr] = singles.tile([128, PW], bf16, name=f"whp{pr}")
            nc.sync.dma_start(out=wh_p[pr][:], in_=wht_ext[:, pr * PW:(pr + 1) * PW])
        bias_t = singles.tile([128, 2 * HC], f32, name="bias_t")
        nc.gpsimd.dma_start(out=bias_t[:], in_=bias_ext[:, :])
        bz_col = [bias_t[:, hc:hc + 1] for hc in range(HC)]
        nbh_col = [bias_t[:, HC + hc:HC + hc + 1] for hc in range(HC)]

        def wchunk(wp, hc, dc):
            o = (hc % 2) * DC * 128 + dc * 128
            return wp[hc // 2][:, o:o + 128]

        # carry[(pair, stream)] = [128,1] fp32 AP with h' of the last
        # processed column for that hc chunk.
        carry = {}

        for b in range(BL):
            for tb in range(NTB):
                if b == 0 and tb == 0:
                    xs, xs8 = x0, x08
                else:
                    # split across queues to parallelize the transfers
                    xs = x_pool.tile([128, DC * TBLK], bf16, tag="x")
                    nc.sync.dma_start(
                        out=xs[:, 0:2 * TBLK], in_=xflat(b, tb, 0, 2)
                    )
                    nc.gpsimd.dma_start(
                        out=xs[:, 2 * TBLK:], in_=xflat(b, tb, 2, 2)
                    )
                    xs8 = x_pool.tile([128, 2 * ZDC * TBLK], f8, tag="x8")
                    nc.sync.dma_start(out=xs8[:], in_=x8flat(b, tb))
                jt = j_pool.tile([128, TBLK], bf16, tag="J")
                iv = itau_ext[b, 0, tb * TBLK:(tb + 1) * TBLK]
                iv_b = bass.AP(
                    tensor=iv.tensor, offset=iv.offset, ap=[[0, 128]] + list(iv.ap)
                )
                nc.gpsimd.dma_start(out=jt[:], in_=iv_b)

                # Emit all z-paths first, then all q-paths: every PSUM
                # buffer handoff then has a full matmul-group (~3.4us) of
                # slack, so the PE never stalls at group boundaries (stalls
                # also drop it out of its 2.4GHz p-state).
                zs = [None] * PAIRS
                for pair in range(PAIRS):
                    hcA, hcB = 2 * pair, 2 * pair + 1

                    # z-preactivations for both streams: 4 psum banks.
                    # Per 512-col half: one fp8 DoubleRow matmul covers
                    # d in [0,256) (K=256 per instruction, 2x rate), then
                    # two scaled-bf16 matmuls cover d in [256,512).
                    zp = psum.tile([128, 2 * TBLK], f32, tag="zq")
                    for s, hc in enumerate((hcA, hcB)):
                        # fp8 DoubleRow chunks (K=256 each), weight-major so
                        # each lhsT loads once per 2 matmuls
                        for dp in range(ZDC):
                            o8 = (hc % 2) * ZDC * 256 + dp * 256
                            w8c = wz8_p[hc // 2][:, o8:o8 + 128]
                            w8_ap = bass.AP(
                                tensor=w8c.tensor, offset=w8c.offset,
                                ap=[list(w8c.ap[0]), [128, 2], [1, 128]],
                            )
                            for half in range(TBLK // MMN):
                                psl = slice(
                                    s * TBLK + half * MMN,
                                    s * TBLK + (half + 1) * MMN,
                                )
                                xo = 2 * dp * TBLK + half * MMN
                                x8c = xs8[:, xo:xo + MMN]
                                x8_ap = bass.AP(
                                    tensor=x8c.tensor, offset=x8c.offset,
                                    ap=[list(x8c.ap[0]), [TBLK, 2], [1, MMN]],
                                )
                                nc.tensor.matmul(
                                    zp[:, psl], lhsT=w8_ap, rhs=x8_ap,
                                    start=(dp == 0),
                                    stop=(FULL_Z_FP8 and dp == ZDC - 1),
                                    perf_mode=mybir.MatmulPerfMode.DoubleRow,
                                )
                        if not FULL_Z_FP8:
                            for dc in (2, 3):
                                wbc = wzb_p[hc // 2][
                                    :, (hc % 2) * 256 + (dc - 2) * 128:
                                    (hc % 2) * 256 + (dc - 1) * 128
                                ]
                                for half in range(TBLK // MMN):
                                    psl = slice(
                                        s * TBLK + half * MMN,
                                        s * TBLK + (half + 1) * MMN,
                                    )
                                    csl = slice(
                                        dc * TBLK + half * MMN,
                                        dc * TBLK + (half + 1) * MMN,
                                    )
                                    nc.tensor.matmul(
                                        zp[:, psl],
                                        lhsT=wbc,
                                        rhs=xs[:, csl],
                                        start=False,
                                        stop=(dc == 3),
                                    )

                    # u = (zpre + bz) * invtau, written column-interleaved
                    u = u_pool.tile([128, 2 * TBLK], bf16, tag="u")
                    for s, hc in enumerate((hcA, hcB)):
                        u_int = bass.AP(
                            tensor=u[:].tensor,
                            offset=u[:].offset + s,
                            ap=[list(u[:].ap[0]), [2, TBLK]],
                        )
                        nc.vector.scalar_tensor_tensor(
                            u_int,
                            zp[:, s * TBLK:(s + 1) * TBLK],
                            bz_col[hc],
                            jt[:],
                            op0=OP.add,
                            op1=OP.mult,
                        )

                    z = z_pool.tile([128, 2 * TBLK], bf16, tag="z")
                    nc.scalar.activation(z[:], u[:], AF.Sigmoid)
                    zs[pair] = z

                for pair in range(PAIRS):
                    hcA, hcB = 2 * pair, 2 * pair + 1

                    # candidate q for both streams; dc order (2,3,0,1) so the
                    # first (b,tb) can start before its x dc0/dc1 chunks land
                    qp = psum.tile([128, 2 * TBLK], f32, tag="zq")
                    for s, hc in enumerate((hcA, hcB)):
                        for idc, dc in enumerate((2, 3, 0, 1)):
                            for half in range(TBLK // MMN):
                                csl = slice(
                                    dc * TBLK + half * MMN,
                                    dc * TBLK + (half + 1) * MMN,
                                )
                                psl = slice(
                                    s * TBLK + half * MMN,
                                    s * TBLK + (half + 1) * MMN,
                                )
                                nc.tensor.matmul(
                                    qp[:, psl],
                                    lhsT=wchunk(wh_p, hc, dc),
                                    rhs=xs[:, csl],
                                    start=(idc == 0),
                                    stop=(idc == DC - 1),
                                )

                    # fused interleaved scan: h' = (1-z) h'_prev + z q
                    qa = qp[:, 0:TBLK]
                    q_pair = bass.AP(
                        tensor=qa.tensor, offset=qa.offset,
                        ap=list(qa.ap) + [[TBLK, 2]],
                    )
                    h = h_pool.tile([128, 2 * TBLK], bf16, tag="h")
                    if tb == 0:
                        s0, s1 = nbh_col[hcA], nbh_col[hcB]
                    else:
                        s0, s1 = carry[(pair, 0)], carry[(pair, 1)]
                    last_group = (
                        b == BL - 1 and tb == NTB - 1 and pair == PAIRS - 1
                    )
                    if not last_group:
                        nc.vector._custom_dve(
                            MINGRU_SCAN,
                            out=h[:], in0=q_pair, in1=zs[pair][:], s0=s0, s1=s1,
                        )
                        if tb < NTB - 1:
                            # custom-DVE scalar reads must be fp32: stage the
                            # last column pair through a small fp32 tile
                            ct = c_pool.tile([128, 2], f32, tag=f"c{pair}")
                            nc.scalar.copy(ct[:], h[:, 2 * TBLK - 2:2 * TBLK])
                            carry[(pair, 0)] = ct[:, 0:1]
                            carry[(pair, 1)] = ct[:, 1:2]
                        nc.scalar.dma_start(out=out_ext[b, pair, tb], in_=h[:])
                    else:
                        # final group: scan + DMA in halves so the out-DMA of
                        # the first half overlaps the second half's scan
                        MH = TBLK // 2
                        qh0 = qp[:, 0:MH]
                        nc.vector._custom_dve(
                            MINGRU_SCAN,
                            out=h[:, 0:TBLK],
                            in0=bass.AP(tensor=qh0.tensor, offset=qh0.offset,
                                        ap=list(qh0.ap) + [[TBLK, 2]]),
                            in1=zs[pair][:, 0:TBLK], s0=s0, s1=s1,
                        )
                        ct = c_pool.tile([128, 2], f32, tag=f"c{pair}")
                        nc.scalar.copy(ct[:], h[:, TBLK - 2:TBLK])
                        nc.scalar.dma_start(
                            out=out_ext[b, pair, tb, :, 0:TBLK], in_=h[:, 0:TBLK]
                        )
                        qh1 = qp[:, MH:2 * MH]
                        nc.vector._custom_dve(
                            MINGRU_SCAN,
                            out=h[:, TBLK:],
                            in0=bass.AP(tensor=qh1.tensor, offset=qh1.offset,
                                        ap=list(qh1.ap) + [[TBLK, 2]]),
                            in1=zs[pair][:, TBLK:],
                            s0=ct[:, 0:1], s1=ct[:, 1:2],
                        )
                        nc.scalar.dma_start(
                            out=out_ext[b, pair, tb, :, TBLK:], in_=h[:, TBLK:]
                        )

    nc.compile()
    return nc


def _prep_inputs(x, motion_mag, Wz, bz, Wh, bh, motion_weight, motion_bias, alpha):
    bf = ml_dtypes.bfloat16
    x = np.asarray(x, dtype=np.float32)
    mm = np.asarray(motion_mag, dtype=np.float32)
    Wz = np.asarray(Wz, dtype=np.float32)
    Wh = np.asarray(Wh, dtype=np.float32)
    bz = np.asarray(bz, dtype=np.float32)
    bh = np.asarray(bh, dtype=np.float32)
    # [128, 2*HC]: bz columns then -bh columns, per hc chunk
    bias = np.concatenate(
        [bz.reshape(HC, 128).T, -bh.reshape(HC, 128).T], axis=1
    ).astype(np.float32)
    mw = float(np.asarray(motion_weight))
    mb = float(np.asarray(motion_bias))
    al = float(np.asarray(alpha))

    f8 = ml_dtypes.float8_e4m3fn
    XS, WS = 16.0, 2048.0            # fp8 scales; combined 2^15
    SC = XS * WS

    a_sp = float(np.log1p(np.exp(al)))  # softplus(alpha)
    sig = 1.0 / (1.0 + np.exp(-(mw * mm + mb)))
    # z-PSUM is 2^15-scaled; descale via invtau, re-scale bz to match
    invtau = ((1.0 / (1.0 + a_sp * sig)) / SC).astype(bf)
    bias[:, 0:HC] *= SC

    WzT = Wz.T  # [D, H]
    D8 = ZDC * 256  # leading contraction depth done in fp8 on the z-path
    # fp8 z-weights, d in [0, D8): [p, hc*(2*ZDC*128) + i*128 + m]
    wz8 = np.ascontiguousarray(
        WzT[0:D8].reshape(2 * ZDC, 128, HC, 128).transpose(1, 2, 0, 3).reshape(
            128, HC * 2 * ZDC * 128))
    wz8 = np.clip(wz8 * WS, -200, 200).astype(f8)
    wht = np.ascontiguousarray(
        Wh.T.reshape(DC, 128, HC, 128).transpose(1, 2, 0, 3).reshape(
            128, HC * DC * 128)).astype(bf)

    in_maps = []
    for c in range(NCORES):
        xl = x[c * BL:(c + 1) * BL]  # [BL, T, D]
        xt = np.ascontiguousarray(
            xl.reshape(BL, NTB, TBLK, DC, 128).transpose(4, 0, 1, 3, 2)
        ).astype(bf)
        x8 = np.ascontiguousarray(
            xl[..., 0:D8].reshape(BL, NTB, TBLK, 2 * ZDC, 128)
            .transpose(4, 0, 1, 3, 2)
        )
        x8 = np.clip(x8 * XS, -200, 200).astype(f8)
        m = {
            "xt": xt,
            "x8": x8,
            "wz8": wz8,
            "wht": wht,
            "bias": bias,
            "invtau": np.ascontiguousarray(
                invtau[c * BL:(c + 1) * BL]).reshape(BL, 1, T),
        }
        if not FULL_Z_FP8:
            # scaled-bf16 z-weights, d in [256,512)
            m["wzt"] = np.ascontiguousarray(
                WzT[256:512].reshape(2, 128, HC, 128).transpose(1, 2, 0, 3)
                .reshape(128, HC * 2 * 128) * SC).astype(bf)
        in_maps.append(m)
    return in_maps, bh


def _assemble(results, bh):
    outs = []
    for c in range(NCORES):
        o = np.asarray(results[c]["out"], dtype=np.float32)
        # [BL, PAIRS, NTB, 128, 2*TBLK] -> [BL, T, H]
        o = o.reshape(BL, PAIRS, NTB, 128, TBLK, 2)
        o = np.transpose(o, (0, 2, 4, 1, 5, 3)).reshape(BL, T, H)
        outs.append(o)
    full = np.ascontiguousarray(np.concatenate(outs, axis=0))
    bhf = bh.reshape(H)
    if np.any(bhf):
        full += bhf
    return full


def _run(inputs, trace=False):
    from concourse.bass_utils import run_bass_kernel_spmd

    if "nc" not in _CACHE:
        _CACHE["nc"] = _build_nc()
    nc = _CACHE["nc"]
    in_maps, bh = _prep_inputs(**inputs)
    res = run_bass_kernel_spmd(nc, in_maps, list(range(NCORES)), trace=trace)
    return _assemble(res.results, bh), res


def kernel(**inputs):
    out, _ = _run(inputs, trace=False)
    return out
